# revision 1
# baseline (speedup 1.0000x reference)
"""Trainium2 Bass kernel for CrossAttentionConditionInjection.

Math: the attention keys/values come from a single condition token broadcast
across the sequence, so the scores are constant along the key axis; softmax is
exactly uniform and the attention output collapses to

    out[b, s, :] = (condition[b] @ Wv.T + bv) @ Wo.T + bo      (for every s)

independent of hidden_states / Wq / Wk / q entirely.

Sharding (2D): core i -> (batch b = i//4, output-column quarter q = i%4).
Every core computes the full v1 = cond[b] @ Wv.T + bv (Wv.T is irreducible
per-core without cross-core exchange) but only its 256-column slice of
row = v1 @ Wo.T + bo, and broadcast-writes it across all 2048 sequence
positions of its batch.  The host reassembles the column quarters.

Engine split (fp32 streams through the PE at ~4 cycles/column, so the
mat-vec bulk stays off the PE):
  stage 1 muls: ACT activation(Copy, scale=cond-per-partition), one per wv
      k-chunk, chasing the chunked wv DMA.
  stage 1 sum:  DVE rolling adds (in-place accumulate), also chasing.
  v1T:          8 tiny PE matmuls  lhsT=partial-chunk, rhs=ones column ->
                v1 landed on partitions; one DVE add folds in bv.
  v1 broadcast: single DVE copy with a step-0 AP.
  stage 2:      8 PE matmuls N=256 over the per-core Wo.T quarter + a K=1
                ones-row matmul for bo.
  output:       one DMA broadcast-writes the [128, 256] row tile 16x into
                the contiguous per-core [2048, 256] output.
"""

import numpy as np
from contextlib import ExitStack

import concourse.bass as bass
import concourse.bacc as bacc
import concourse.mybir as mybir
import concourse.tile as tile
from concourse.bass_utils import run_bass_kernel_spmd

B, S, D = 2, 2048, 1024
NCORES = 8
QCORES = NCORES // B  # cores per batch -> column quarters
QW = D // QCORES  # 256 columns per core
KC = D // 128  # 8 contraction chunks
WV_CHUNKS = 8

_cache = {}


def _build():
    f32 = mybir.dt.float32
    nc = bacc.Bacc()

    smalls = nc.dram_tensor("smalls", [128, 2 * KC], f32, kind="ExternalInput")
    wvp = nc.dram_tensor("wvp", [128, KC * D], f32, kind="ExternalInput")
    woq = nc.dram_tensor("woq", [128, KC * QW], f32, kind="ExternalInput")
    boq = nc.dram_tensor("boq", [1, QW], f32, kind="ExternalInput")
    y = nc.dram_tensor("y", [128, (S // 128) * QW], f32, kind="ExternalOutput")

    with tile.TileContext(nc) as tc, ExitStack() as ctx:
        wv_pool = ctx.enter_context(tc.tile_pool(name="wv", bufs=1))
        wo_pool = ctx.enter_context(tc.tile_pool(name="wo", bufs=1))
        small = ctx.enter_context(tc.tile_pool(name="small", bufs=1))
        tmpp = ctx.enter_context(tc.tile_pool(name="tmpp", bufs=1))
        outp = ctx.enter_context(tc.tile_pool(name="outp", bufs=1))
        psumv = ctx.enter_context(
            tc.tile_pool(name="psumv", bufs=1, space=bass.MemorySpace.PSUM)
        )
        psum2 = ctx.enter_context(
            tc.tile_pool(name="psum2", bufs=1, space=bass.MemorySpace.PSUM)
        )

        ones1x128 = small.tile([1, 128], f32)
        nc.vector.memset(ones1x128[:], 1.0)
        ones_col = small.tile([128, 1], f32)
        nc.vector.memset(ones_col[:], 1.0)

        from concourse.tile_rust import add_dep_helper

        # ---- loads: smalls, then wv split across both HWDGE rings.
        # Issue order is forced with dep edges: data chunks in k order first
        # (the ring drains FIFO), woq/boq at the tail.
        smalls_sb = small.tile([128, 2 * KC], f32)
        sync_prev = nc.sync.dma_start(smalls_sb[:], smalls[:])
        condT = smalls_sb[:, 0:KC]
        bvT = smalls_sb[:, KC : 2 * KC]

        wv_sb = wv_pool.tile([128, KC * D], f32)
        wv_c = (KC * D) // WV_CHUNKS
        scalar_prev = None
        for c in range(WV_CHUNKS):
            eng = nc.sync if c % 2 == 0 else nc.scalar
            dma = eng.dma_start(
                wv_sb[:, c * wv_c : (c + 1) * wv_c], wvp[:, c * wv_c : (c + 1) * wv_c]
            )
            if c % 2 == 0:
                add_dep_helper(dma.ins, sync_prev.ins, sync=False, reason="ring order")
                sync_prev = dma
            else:
                if scalar_prev is not None:
                    add_dep_helper(dma.ins, scalar_prev.ins, sync=False, reason="ring order")
                scalar_prev = dma
        woq_sb = wo_pool.tile([128, KC * QW], f32)
        dma = nc.sync.dma_start(woq_sb[:], woq[:])
        add_dep_helper(dma.ins, sync_prev.ins, sync=False, reason="ring order")
        sync_prev = dma
        boq_sb = small.tile([1, QW], f32)
        dma = nc.sync.dma_start(boq_sb[:], boq[:])
        add_dep_helper(dma.ins, sync_prev.ins, sync=False, reason="ring order")

        # ---- stage 1: partial[p, c] = sum_k WvT[k*128+p, c] * cond[k*128+p]
        # DVE does mul0 + the rolling sum (tensor_scalar is 2x on DVE);
        # ACT does muls 1..7, explicitly chained in chunk order (the ACT
        # queue is strict FIFO -- an unordered schedule head-of-line blocks
        # on a late chunk).  Odd-chunk DMA issues ride the ACT ring between
        # the multiplies; they are nonblocking.
        from concourse.tile_rust import add_dep_helper

        tmp = tmpp.tile([128, KC * D], f32)
        partial = tmp[:, :D]
        prev_mul = None
        nc.vector.tensor_scalar_mul(tmp[:, 0:D], wv_sb[:, 0:D], condT[:, 0:1])
        for k in range(1, KC):
            m = nc.scalar.activation(
                tmp[:, k * D : (k + 1) * D],
                wv_sb[:, k * D : (k + 1) * D],
                mybir.ActivationFunctionType.Copy,
                scale=condT[:, k : k + 1],
            )
            if prev_mul is not None:
                add_dep_helper(m.ins, prev_mul.ins, sync=True, reason="mul chunk order")
            prev_mul = m
        for k in range(1, KC - 1):
            nc.vector.tensor_add(partial, partial, tmp[:, k * D : (k + 1) * D])
        lk = KC - 1
        nc.vector.tensor_add(
            partial[:, 0:512], partial[:, 0:512], tmp[:, lk * D : lk * D + 512]
        )
        nc.vector.tensor_add(
            partial[:, 512:D], partial[:, 512:D], tmp[:, lk * D + 512 : (lk + 1) * D]
        )


        # ---- v1T on partitions: out[m, 0] = sum_p partial[p, j*128+m] ----
        v1T_ps = psumv.tile([128, KC], f32)
        for j in range(KC):
            nc.tensor.matmul(
                v1T_ps[:, j : j + 1],
                partial[:, j * 128 : (j + 1) * 128],
                ones_col[:],
                start=True,
                stop=True,
            )
        v1T_sb = small.tile([128, KC], f32)
        nc.vector.tensor_add(v1T_sb[:], v1T_ps[:], bvT[:])

        # broadcast each v1T column across the free dim for stage-2 lhsT
        v1bc_sb = small.tile([128, D], f32)
        nc.vector.tensor_copy(
            v1bc_sb[:].rearrange("p (k m) -> p k m", k=KC),
            v1T_sb[:, :, None].broadcast_to([128, KC, 128]),
        )

        # ---- stage 2 on PE: out quarter (broadcast) = v1 @ WoT[:, q] + bo ----
        acc = psum2.tile([128, QW], f32)
        for k in range(KC):
            nc.tensor.matmul(
                acc[:],
                v1bc_sb[:, k * 128 : (k + 1) * 128],
                woq_sb[:, k * QW : (k + 1) * QW],
                start=(k == 0),
                stop=False,
            )
        nc.tensor.matmul(acc[:], ones1x128[:], boq_sb[:], start=False, stop=True)
        out_sb = outp.tile([128, QW], f32)
        nc.vector.tensor_copy(out_sb[:], acc[:])

        # ---- broadcast-write the row quarter, contiguous per partition ----
        nc.sync.dma_start(
            y[:].rearrange("p (a c) -> p a c", a=S // 128),
            out_sb[:, None, :].broadcast_to([128, S // 128, QW]),
        )

    nc.compile()
    return nc


def _prep_inputs(condition, Wv, bv, Wo, bo):
    cond = np.asarray(condition, np.float32)
    wvt = np.ascontiguousarray(
        np.asarray(Wv, np.float32).T.reshape(KC, 128, D).transpose(1, 0, 2).reshape(128, KC * D)
    )
    WoT = np.asarray(Wo, np.float32).T  # [d, dout]
    bvT = np.asarray(bv, np.float32).reshape(KC, 128).T  # [128, KC]
    bo_ = np.asarray(bo, np.float32)
    smalls = []
    for b in range(B):
        condT = cond[b].reshape(KC, 128).T  # [128, KC]
        smalls.append(np.ascontiguousarray(np.concatenate([condT, bvT], axis=1)))
    woqs, boqs = [], []
    for q in range(QCORES):
        sl = WoT[:, q * QW : (q + 1) * QW]  # [1024, 256]
        woqs.append(
            np.ascontiguousarray(
                sl.reshape(KC, 128, QW).transpose(1, 0, 2).reshape(128, KC * QW)
            )
        )
        boqs.append(np.ascontiguousarray(bo_[q * QW : (q + 1) * QW].reshape(1, QW)))
    in_maps = []
    for i in range(NCORES):
        b, q = i // QCORES, i % QCORES
        in_maps.append(
            {"smalls": smalls[b], "wvp": wvt, "woq": woqs[q], "boq": boqs[q]}
        )
    return in_maps


def _run(in_maps, **kwargs):
    if "nc" not in _cache:
        _cache["nc"] = _build()
    return run_bass_kernel_spmd(
        _cache["nc"], in_maps, core_ids=list(range(NCORES)), **kwargs
    )


def kernel(hidden_states, condition, Wq, bq, Wk, bk, Wv, bv, Wo, bo):
    in_maps = _prep_inputs(condition, Wv, bv, Wo, bo)
    res = _run(in_maps)
    full = np.empty((B, S, D), np.float32)
    for i in range(NCORES):
        b, q = i // QCORES, i % QCORES
        yv = np.asarray(res.results[i]["y"]).reshape(128, S // 128, QW)
        full[b, :, q * QW : (q + 1) * QW] = (
            yv.transpose(1, 0, 2).reshape(S, QW)
        )
    return full



# revision 7
# speedup vs baseline: 1.7747x; 1.7747x over previous
"""Trainium2 Bass kernel for CrossAttentionConditionInjection.

Math: the attention keys/values come from a single condition token broadcast
across the sequence, so the scores are constant along the key axis; softmax is
exactly uniform and the attention output collapses to

    out[b, s, :] = (condition[b] @ Wv.T + bv) @ Wo.T + bo      (for every s)

independent of hidden_states / Wq / Wk / q entirely.

Sharding: core i owns output columns [128*i, 128*(i+1)) for BOTH batches.
Weights stream in bf16 (rel-err budget 2e-2 >> bf16's ~4e-3), halving the
dominant Wv load.  All matvec work runs on the PE as stationary-weight
matmuls chasing the chunked DMA stream:

  stage 1: per j-chunk jc, 8 matmuls  lhsT=WvT block [128k,128j] (stationary),
           rhs=condT k-slice [128k, 2b] -> v1T_ps[:, jc*2:jc*2+2]  (accum over k)
  bv add:  per jc a tiny DVE add (+cast to bf16) -> v1T_sb
  stage 2: 8 matmuls lhsT=v1T_sb jc-slice [128j, 2], rhs=WoT block [128j, 128n]
           -> row_ps [2b, 128n]; a [1,2]x[1,128] ones-matmul folds in bo.
  bcast:   2 matmuls lhsT=batch-indicator [2, 128], rhs=row bf16 [2, 128]
           -> out_ps [128 (all partitions = copies), 128n] per batch.
  write:   2 DMAs (one per HWDGE ring), each broadcast-writing its batch's
           [128, 128] row tile 16x into the contiguous per-core [2048, 128]
           output quarter-column.

A dozen dummy matmuls head the PE queue to lift the HAM clock gate
(1.2 -> 2.4 GHz) while the first Wv chunks are still in flight.
"""

import numpy as np
from contextlib import ExitStack

import ml_dtypes

import concourse.bass as bass
import concourse.bacc as bacc
import concourse.mybir as mybir
import concourse.tile as tile
from concourse.bass_utils import run_bass_kernel_spmd

B, S, D = 2, 2048, 1024
NCORES = 8
NW = D // NCORES  # 128 output columns per core
KC = D // 128  # 8 contraction chunks (k)
JC = D // 128  # 8 v1 chunks (j)
SA = S // 128  # 16 sequence blocks
BF16 = ml_dtypes.bfloat16

_cache = {}


def _build():
    f32 = mybir.dt.float32
    bf16 = mybir.dt.bfloat16
    nc = bacc.Bacc()

    condT = nc.dram_tensor("condT", [128, KC * B], bf16, kind="ExternalInput")
    bvT = nc.dram_tensor("bvT", [128, JC], f32, kind="ExternalInput")
    bo_t = nc.dram_tensor("bo", [1, NW], bf16, kind="ExternalInput")
    ebs_t = nc.dram_tensor("ebs", [B, B * NW], bf16, kind="ExternalInput")
    wv = nc.dram_tensor("wv", [128, JC * KC * 128], bf16, kind="ExternalInput")
    wo = nc.dram_tensor("wo", [128, JC * NW], bf16, kind="ExternalInput")
    y = nc.dram_tensor("y", [128, B * SA * NW], f32, kind="ExternalOutput")

    with tile.TileContext(nc) as tc, ExitStack() as ctx:
        wv_pool = ctx.enter_context(tc.tile_pool(name="wv", bufs=1))
        wo_pool = ctx.enter_context(tc.tile_pool(name="wo", bufs=1))
        small = ctx.enter_context(tc.tile_pool(name="small", bufs=1))
        outp = ctx.enter_context(tc.tile_pool(name="outp", bufs=1))
        ps_warm = ctx.enter_context(
            tc.tile_pool(name="ps_warm", bufs=1, space=bass.MemorySpace.PSUM)
        )
        ps_v1 = ctx.enter_context(
            tc.tile_pool(name="ps_v1", bufs=1, space=bass.MemorySpace.PSUM)
        )
        ps_row = ctx.enter_context(
            tc.tile_pool(name="ps_row", bufs=1, space=bass.MemorySpace.PSUM)
        )
        ps_out = ctx.enter_context(
            tc.tile_pool(name="ps_out", bufs=1, space=bass.MemorySpace.PSUM)
        )

        from concourse.tile_rust import add_dep_helper

        # small constants
        ones2 = small.tile([1, 2], bf16)
        nc.vector.memset(ones2[:], 1.0)
        warm = small.tile([128, 128], bf16)
        nc.vector.memset(warm[:], 0.0)
        ebs = small.tile([B, B * NW], bf16)  # batch indicators for bcast (host)

        # ---- loads. ring A = nc.sync, ring B = nc.scalar; order forced with
        # dep edges (each HWDGE ring drains FIFO).
        condT_sb = small.tile([128, KC * B], bf16)
        bvT_sb = small.tile([128, JC], f32)
        bo_sb = small.tile([1, NW], bf16)
        wv_sb = wv_pool.tile([128, JC * KC * 128], bf16)
        wo_sb = wo_pool.tile([128, JC * NW], bf16)

        jcw = KC * 128  # free-dim columns per wv j-chunk

        prev_a = nc.sync.dma_start(condT_sb[:], condT[:])
        d = nc.sync.dma_start(bvT_sb[:], bvT[:])
        add_dep_helper(d.ins, prev_a.ins, sync=False, reason="ringA order")
        prev_a = d
        prev_b = nc.scalar.dma_start(bo_sb[:], bo_t[:])
        d = nc.scalar.dma_start(ebs[:], ebs_t[:])
        add_dep_helper(d.ins, prev_b.ins, sync=False, reason="ringB order")
        prev_b = d
        wv_dma = [None] * JC
        for jc in range(JC):
            eng = nc.sync if jc % 2 == 0 else nc.scalar
            d = eng.dma_start(
                wv_sb[:, jc * jcw : (jc + 1) * jcw], wv[:, jc * jcw : (jc + 1) * jcw]
            )
            if jc % 2 == 0:
                add_dep_helper(d.ins, prev_a.ins, sync=False, reason="ringA order")
                prev_a = d
            else:
                add_dep_helper(d.ins, prev_b.ins, sync=False, reason="ringB order")
                prev_b = d
            wv_dma[jc] = d
        # wo halves ride the ring tails (needed only by stage 2)
        woh = JC * NW // 2
        d = nc.scalar.dma_start(wo_sb[:, 0:woh], wo[:, 0:woh])
        add_dep_helper(d.ins, prev_b.ins, sync=False, reason="ringB order")
        prev_b = d
        d = nc.sync.dma_start(wo_sb[:, woh:], wo[:, woh:])
        add_dep_helper(d.ins, prev_a.ins, sync=False, reason="ringA order")
        prev_a = d

        # ---- PE warmup: lift the HAM clock gate while DMA streams in ----
        warm_ps = ps_warm.tile([128, 128], f32)
        for _ in range(12):
            nc.tensor.matmul(warm_ps[:], warm[:], warm[:], start=True, stop=True)

        # ---- stage 1: v1T[:, jc*2+b] = sum_k WvT[k, jc*128+p] cond[b, k] ----
        v1T_ps = ps_v1.tile([128, JC * B], f32)
        v1T_sb = small.tile([128, JC * B], bf16)
        for jc in range(JC):
            for kc in range(KC):
                nc.tensor.matmul(
                    v1T_ps[:, jc * B : (jc + 1) * B],
                    wv_sb[:, (jc * KC + kc) * 128 : (jc * KC + kc + 1) * 128],
                    condT_sb[:, kc * B : (kc + 1) * B],
                    start=(kc == 0),
                    stop=(kc == KC - 1),
                )
            # fold in bv, cast to bf16 for stage 2
            nc.vector.tensor_add(
                v1T_sb[:, jc * B : (jc + 1) * B],
                v1T_ps[:, jc * B : (jc + 1) * B],
                bvT_sb[:, jc : jc + 1].broadcast_to([128, B]),
            )

        # ---- stage 2: row[b, n] = sum_j v1[b, j] WoT[j, n]  (+ bo) ----
        row_ps = ps_row.tile([B, NW], f32)
        for jc in range(JC):
            nc.tensor.matmul(
                row_ps[:],
                v1T_sb[:, jc * B : (jc + 1) * B],
                wo_sb[:, jc * NW : (jc + 1) * NW],
                start=(jc == 0),
                stop=False,
            )
        nc.tensor.matmul(row_ps[:], ones2[:], bo_sb[:], start=False, stop=True)
        row_sb = small.tile([B, NW], bf16)
        nc.vector.tensor_copy(row_sb[:], row_ps[:])

        # ---- broadcast across partitions (seq positions) per batch ----
        out_ps = ps_out.tile([128, B * NW], f32)
        for b in range(B):
            nc.tensor.matmul(
                out_ps[:, b * NW : (b + 1) * NW],
                ebs[:, b * NW : (b + 1) * NW],
                row_sb[:],
                start=True,
                stop=True,
            )
        out_sb = outp.tile([128, B * NW], f32)
        nc.vector.tensor_copy(out_sb[:], out_ps[:])

        # ---- broadcast-write: per batch one DMA, per-partition contiguous ----
        for b in range(B):
            eng = nc.sync if b == 0 else nc.scalar
            d = eng.dma_start(
                y[:, b * SA * NW : (b + 1) * SA * NW].rearrange(
                    "p (a c) -> p a c", a=SA
                ),
                out_sb[:, b * NW : (b + 1) * NW][:, None, :].broadcast_to(
                    [128, SA, NW]
                ),
            )
            prev = prev_a if b == 0 else prev_b
            add_dep_helper(d.ins, prev.ins, sync=False, reason="ring order")

    nc.compile()
    return nc


def _prep_inputs(condition, Wv, bv, Wo, bo):
    cond = np.asarray(condition, np.float32)
    Wv = np.asarray(Wv, np.float32)
    Wo = np.asarray(Wo, np.float32)
    bv = np.asarray(bv, np.float32)
    bo = np.asarray(bo, np.float32)

    # wv[p, ((jc*KC+kc)*128)+c] = Wv[jc*128+c, kc*128+p]
    wv_host = np.ascontiguousarray(
        Wv.reshape(JC, 128, KC, 128).transpose(3, 0, 2, 1).reshape(128, JC * KC * 128)
    ).astype(BF16)
    # condT[p, kc*B+b] = cond[b, kc*128+p]
    condT = np.ascontiguousarray(
        cond.T.reshape(KC, 128, B).transpose(1, 0, 2).reshape(128, KC * B)
    ).astype(BF16)
    bvT = np.ascontiguousarray(bv.reshape(JC, 128).T)
    ebs = np.zeros((B, B * NW), BF16)
    for b in range(B):
        ebs[b, b * NW : (b + 1) * NW] = 1.0

    in_maps = []
    for i in range(NCORES):
        # wo[p, jc*NW+c] = Wo[i*NW+c, jc*128+p]
        wo_i = np.ascontiguousarray(
            Wo[i * NW : (i + 1) * NW]
            .reshape(NW, JC, 128)
            .transpose(2, 1, 0)
            .reshape(128, JC * NW)
        ).astype(BF16)
        bo_i = np.ascontiguousarray(bo[i * NW : (i + 1) * NW].reshape(1, NW)).astype(
            BF16
        )
        in_maps.append(
            {"condT": condT, "bvT": bvT, "bo": bo_i, "ebs": ebs, "wv": wv_host, "wo": wo_i}
        )
    return in_maps


def _run(in_maps, **kwargs):
    if "nc" not in _cache:
        _cache["nc"] = _build()
    return run_bass_kernel_spmd(
        _cache["nc"], in_maps, core_ids=list(range(NCORES)), **kwargs
    )


def kernel(hidden_states, condition, Wq, bq, Wk, bk, Wv, bv, Wo, bo):
    in_maps = _prep_inputs(condition, Wv, bv, Wo, bo)
    res = _run(in_maps)
    full = np.empty((B, S, D), np.float32)
    for i in range(NCORES):
        yv = np.asarray(res.results[i]["y"]).reshape(128, B, SA, NW)
        full[:, :, i * NW : (i + 1) * NW] = (
            yv.transpose(1, 2, 0, 3).reshape(B, S, NW)
        )
    return full
